# revision 8
# baseline (speedup 1.0000x reference)
"""CMN (collaborative memory network) forward on 8 TRN2 NeuronCores.

Strategy: data-parallel over the pair batch (16384 pairs/core); embedding
tables replicated per core.  The irregular 256B-row gathers are done with
chunked int16 dma_gather (4 SWDGE queues) into a DRAM staging area laid out
in per-block regions, then one dma_gather per 2048-pair block re-gathers the
staged rows in compute order.  All index planning happens on host (numpy);
all row-data movement and math happen on device.
"""
import numpy as np

import concourse.bass as bass
import concourse.bacc as bacc
import concourse.tile as tile
from concourse import mybir
from concourse.bass_utils import run_bass_kernel_spmd
from concourse.library_config import mlp

# problem constants
N_PAIRS = 131072
NUM_USERS = 1_000_000
NUM_ITEMS = 500_000
D = 64
S = 10
N_CORES = 8
PADF = float(np.float32(-2.0 ** 32 + 1))   # == -4294967296.0 in fp32

CHUNK = 32768                               # int16-addressable table window
F32 = mybir.dt.float32
I16 = mybir.dt.int16


def _wrap16(a):
    """[L] int16 -> [128, L//16]: index j at (j%16, j//16), replicated x8."""
    return np.tile(a.reshape(-1, 16).T, (8, 1)).copy()


def _plan_core(users_c, items_c, adjidx_c, npc, nb, padu, padi, nuc, nic):
    """Build per-core phase-1/phase-2 index arrays.

    Returns (p1u [nuc, nb*padu] int16, p1i [nic, nb*padi] int16,
             p2 [nb, 12*pb] int16).
    """
    pb = npc // nb
    region_items_base = nuc * padu
    p1u = np.zeros((nuc, nb * padu), dtype=np.int16)
    p1i = np.zeros((nic, nb * padi), dtype=np.int16)
    p2 = np.zeros((nb, 12 * pb), dtype=np.int16)
    for b in range(nb):
        sl = slice(b * pb, (b + 1) * pb)
        # user-table rows, s-major: s=0..9 neighbours, s=10 the user itself
        u_rows = np.concatenate(
            [adjidx_c[sl, s] for s in range(S)] + [users_c[sl]]).astype(np.int64)
        c_u = u_rows // CHUNK
        loc_u = (u_rows % CHUNK).astype(np.int16)
        perm = np.argsort(c_u, kind="stable")
        counts = np.bincount(c_u, minlength=nuc)
        starts = np.zeros(nuc, dtype=np.int64)
        starts[1:] = np.cumsum(counts)[:-1]
        rank = np.empty(len(u_rows), dtype=np.int64)
        rank[perm] = np.arange(len(u_rows)) - starts[c_u[perm]]
        p2[b, : 11 * pb] = (c_u * padu + rank).astype(np.int16)
        loc_sorted = loc_u[perm]
        for c in range(nuc):
            n = counts[c]
            p1u[c, b * padu: b * padu + n] = loc_sorted[starts[c]: starts[c] + n]
        # item rows, s=11
        i_rows = items_c[sl].astype(np.int64)
        c_i = i_rows // CHUNK
        loc_i = (i_rows % CHUNK).astype(np.int16)
        permi = np.argsort(c_i, kind="stable")
        countsi = np.bincount(c_i, minlength=nic)
        startsi = np.zeros(nic, dtype=np.int64)
        startsi[1:] = np.cumsum(countsi)[:-1]
        ranki = np.empty(len(i_rows), dtype=np.int64)
        ranki[permi] = np.arange(len(i_rows)) - startsi[c_i[permi]]
        p2[b, 11 * pb:] = (region_items_base + c_i * padi + ranki).astype(np.int16)
        loci_sorted = loc_i[permi]
        for c in range(nic):
            n = countsi[c]
            p1i[c, b * padi: b * padi + n] = loci_sorted[startsi[c]: startsi[c] + n]
    return p1u, p1i, p2


def _round128(x):
    return int(-(-x // 128) * 128)


def plan(users, items, adjidx, n_cores, npc, nb):
    """Global planning: pads shared across cores; per-core index arrays."""
    nuc = -(-NUM_USERS // CHUNK)
    nic = -(-NUM_ITEMS // CHUNK)
    pb = npc // nb
    # counts for pad sizing
    maxu, maxi = 0, 0
    for k in range(n_cores):
        sl = slice(k * npc, (k + 1) * npc)
        a = adjidx[sl]
        u = users[sl]
        it = items[sl]
        for b in range(nb):
            bsl = slice(b * pb, (b + 1) * pb)
            rows = np.concatenate(
                [a[bsl, s] for s in range(S)] + [u[bsl]]) // CHUNK
            maxu = max(maxu, int(np.bincount(rows, minlength=nuc).max()))
            maxi = max(maxi, int(np.bincount(it[bsl] // CHUNK,
                                             minlength=nic).max()))
    padu = _round128(maxu)
    padi = _round128(maxi)
    region = nuc * padu + nic * padi
    assert region <= 32767, f"staging region {region} exceeds int16 range"
    cores = []
    for k in range(n_cores):
        sl = slice(k * npc, (k + 1) * npc)
        cores.append(_plan_core(users[sl], items[sl], adjidx[sl],
                                npc, nb, padu, padi, nuc, nic))
    return dict(padu=padu, padi=padi, region=region, nuc=nuc, nic=nic,
                npc=npc, nb=nb, pb=pb, cores=cores)


def build_program(pl):
    """Emit the Bass program for one core (SPMD-shared across cores)."""
    npc, nb, pb = pl["npc"], pl["nb"], pl["pb"]
    nuc, nic = pl["nuc"], pl["nic"]
    padu, padi, region = pl["padu"], pl["padi"], pl["region"]
    t_per = pb // 128                       # t columns per block
    nuidx = nb * padu                       # idxs per user chunk-gather
    niidx = nb * padi
    n2idx = 12 * pb                         # idxs per block re-gather

    nc = bacc.Bacc(None, target_bir_lowering=False, num_swdge_queues=4)
    t_ut = nc.dram_tensor("ut", [NUM_USERS, D], F32, kind="ExternalInput")
    t_it = nc.dram_tensor("it", [NUM_ITEMS, D], F32, kind="ExternalInput")
    t_p1u = nc.dram_tensor("p1u", [nuc, 128, nuidx // 16], I16,
                           kind="ExternalInput")
    t_p1i = nc.dram_tensor("p1i", [nic, 128, niidx // 16], I16,
                           kind="ExternalInput")
    t_p2 = nc.dram_tensor("p2", [nb, 128, n2idx // 16], I16,
                          kind="ExternalInput")
    t_uw = nc.dram_tensor("uw", [128, D], F32, kind="ExternalInput")
    t_vb = nc.dram_tensor("vb", [D, 1], F32, kind="ExternalInput")
    t_v = nc.dram_tensor("v", [D, 1], F32, kind="ExternalInput")
    t_ident = nc.dram_tensor("ident", [128, 128], F32, kind="ExternalInput")
    t_out = nc.dram_tensor("out", [npc], F32, kind="ExternalOutput")

    rr = [0]

    def qn():
        rr[0] = (rr[0] + 1) % 4
        return rr[0]

    GMAX = 8192                             # HW limit per dma_gather inst

    def gather_split(nc_, out_tile, in_ap, idx_tile, total):
        """dma_gather in <=GMAX-index slices (HW rejects large num_idxs)."""
        n_sl = -(-total // GMAX)
        for k in range(n_sl):
            n_k = min(GMAX, total - k * GMAX)
            nc_.gpsimd.dma_gather(
                out_ap=out_tile[:, k * (GMAX // 128):
                                k * (GMAX // 128) + n_k // 128, :],
                in_ap=in_ap,
                idxs_ap=idx_tile[:, k * (GMAX // 16):
                                 k * (GMAX // 16) + n_k // 16],
                num_idxs=n_k, num_idxs_reg=n_k, elem_size=D,
                single_packet=False, queue_num=qn())

    with tile.TileContext(nc) as tc:
        with tc.tile_pool(name="consts", bufs=1) as consts, \
             tc.tile_pool(name="dram", bufs=1, space="DRAM") as dram, \
             tc.tile_pool(name="p1g", bufs=2) as p1g, \
             tc.tile_pool(name="idxp", bufs=2) as idxp, \
             tc.tile_pool(name="ad", bufs=2) as adp, \
             tc.tile_pool(name="prodp", bufs=1) as prodp, \
             tc.tile_pool(name="small", bufs=1) as small, \
             tc.tile_pool(name="tailp", bufs=2) as tailp, \
             tc.tile_pool(name="ps", bufs=2, space="PSUM") as psp:
            nc.gpsimd.load_library(mlp)

            ident = consts.tile([128, 128], F32)
            nc.sync.dma_start(out=ident[:], in_=t_ident[:, :])
            uw_sb = consts.tile([128, D], F32)
            nc.sync.dma_start(out=uw_sb[:], in_=t_uw[:, :])
            vb_sb = consts.tile([D, 1], F32)
            nc.sync.dma_start(out=vb_sb[:], in_=t_vb[:, :])
            v_sb = consts.tile([D, 1], F32)
            nc.sync.dma_start(out=v_sb[:], in_=t_v[:, :])

            stag = dram.tile([nb * region, D], F32)

            # ---- phase 1: chunk-gather table rows into staging regions ----
            for c in range(nuc):
                ic = idxp.tile([128, nuidx // 16], I16, tag="idxu")
                nc.sync.dma_start(out=ic[:], in_=t_p1u[c])
                g = p1g.tile([128, nuidx // 128, D], F32, tag="gu")
                nrows = min(CHUNK, NUM_USERS - c * CHUNK)
                gather_split(nc, g, t_ut[c * CHUNK: c * CHUNK + nrows, :],
                             ic, nuidx)
                for b in range(nb):
                    dst = stag[b * region + c * padu:
                               b * region + (c + 1) * padu, :]
                    nc.sync.dma_start(
                        out=dst.rearrange("(k p) d -> p k d", p=128),
                        in_=g[:, b * (padu // 128): (b + 1) * (padu // 128), :])
            ibase = nuc * padu
            for c in range(nic):
                ic = idxp.tile([128, niidx // 16], I16, tag="idxi")
                nc.sync.dma_start(out=ic[:], in_=t_p1i[c])
                g = p1g.tile([128, niidx // 128, D], F32, tag="gi")
                nrows = min(CHUNK, NUM_ITEMS - c * CHUNK)
                gather_split(nc, g, t_it[c * CHUNK: c * CHUNK + nrows, :],
                             ic, niidx)
                for b in range(nb):
                    dst = stag[b * region + ibase + c * padi:
                               b * region + ibase + (c + 1) * padi, :]
                    nc.sync.dma_start(
                        out=dst.rearrange("(k p) d -> p k d", p=128),
                        in_=g[:, b * (padi // 128): (b + 1) * (padi // 128), :])

            # ---- phase 2 + compute, per block ----
            for b in range(nb):
                i2 = idxp.tile([128, n2idx // 16], I16, tag="idx2")
                nc.sync.dma_start(out=i2[:], in_=t_p2[b])
                ad = adp.tile([128, 12 * t_per, D], F32, tag="ad")
                gather_split(nc, ad, stag[b * region: (b + 1) * region, :],
                             i2, n2idx)

                # views: cols (s, t) s-major; s=10 -> M, s=11 -> I
                M = ad[:, 10 * t_per: 11 * t_per, :]        # [128, T, 64]
                I = ad[:, 11 * t_per: 12 * t_per, :]
                mi = small.tile([128, t_per, D], F32, tag="mi")
                nc.vector.tensor_tensor(out=mi[:], in0=M, in1=I,
                                        op=mybir.AluOpType.add)
                mip = small.tile([128, t_per, D], F32, tag="mip")
                nc.vector.tensor_tensor(out=mip[:], in0=M, in1=I,
                                        op=mybir.AluOpType.mult)

                qt = small.tile([128, S, t_per], F32, tag="qt")
                prod = prodp.tile([128, 5, t_per, D], F32, tag="prod")
                for h in range(2):
                    adj_h = ad[:, h * 5 * t_per: (h + 1) * 5 * t_per, :] \
                        .rearrange("p (s t) d -> p s t d", s=5)
                    mi_b = mi[:].unsqueeze(1).to_broadcast([128, 5, t_per, D])
                    nc.vector.tensor_tensor(out=prod[:], in0=adj_h, in1=mi_b,
                                            op=mybir.AluOpType.mult)
                    nc.vector.tensor_reduce(
                        out=qt[:, h * 5: (h + 1) * 5, :], in_=prod[:],
                        axis=mybir.AxisListType.X, op=mybir.AluOpType.add)

                # q = 10*qt + (qt==0)*PAD
                mk = small.tile([128, S, t_per], F32, tag="mk")
                nc.vector.tensor_scalar(out=mk[:], in0=qt[:], scalar1=0.0,
                                        scalar2=None,
                                        op0=mybir.AluOpType.is_equal)
                nc.vector.tensor_scalar_mul(mk[:], mk[:], PADF)
                q2 = small.tile([128, S, t_per], F32, tag="q2")
                nc.vector.scalar_tensor_tensor(
                    out=q2[:], in0=qt[:], scalar=float(S), in1=mk[:],
                    op0=mybir.AluOpType.mult, op1=mybir.AluOpType.add)

                # softmax over s
                rmax = small.tile([128, t_per], F32, tag="rmax")
                nc.vector.tensor_reduce(
                    out=rmax[:], in_=q2[:].transpose([0, 2, 1]),
                    axis=mybir.AxisListType.X, op=mybir.AluOpType.max)
                nc.vector.tensor_scalar_mul(rmax[:], rmax[:], -1.0)
                ex = small.tile([128, S, t_per], F32, tag="ex")
                nc.vector.tensor_tensor(
                    out=ex[:], in0=q2[:],
                    in1=rmax[:].unsqueeze(1).to_broadcast([128, S, t_per]),
                    op=mybir.AluOpType.add)
                nc.scalar.activation(out=ex[:], in_=ex[:],
                                     func=mybir.ActivationFunctionType.Exp)
                den = small.tile([128, t_per], F32, tag="den")
                nc.vector.tensor_reduce(
                    out=den[:], in_=ex[:].transpose([0, 2, 1]),
                    axis=mybir.AxisListType.X, op=mybir.AluOpType.add)
                nc.vector.reciprocal(out=den[:], in_=den[:])
                w = small.tile([128, S, t_per], F32, tag="w")
                nc.vector.tensor_tensor(
                    out=w[:], in0=ex[:],
                    in1=den[:].unsqueeze(1).to_broadcast([128, S, t_per]),
                    op=mybir.AluOpType.mult)

                # o = sum_s w_s * adj_s
                o = small.tile([128, t_per, D], F32, tag="o")
                oh = small.tile([128, t_per, D], F32, tag="oh")
                for h in range(2):
                    adj_h = ad[:, h * 5 * t_per: (h + 1) * 5 * t_per, :] \
                        .rearrange("p (s t) d -> p s t d", s=5)
                    w_b = w[:, h * 5: (h + 1) * 5, :].unsqueeze(3) \
                        .to_broadcast([128, 5, t_per, D])
                    nc.vector.tensor_tensor(out=prod[:], in0=adj_h, in1=w_b,
                                            op=mybir.AluOpType.mult)
                    dstt = o if h == 0 else oh
                    nc.vector.tensor_reduce(
                        out=dstt[:], in_=prod[:].transpose([0, 2, 3, 1]),
                        axis=mybir.AxisListType.X, op=mybir.AluOpType.add)
                nc.vector.tensor_tensor(out=o[:], in0=o[:], in1=oh[:],
                                        op=mybir.AluOpType.add)

                # tail: pre = [mip; o] matmul, lrelu(+bias), v-dot
                for t in range(t_per):
                    stka = psp.tile([D, 128], F32, tag="stka")
                    nc.tensor.transpose(out=stka[:], in_=mip[:, t, :],
                                        identity=ident[:])
                    stkb = psp.tile([D, 128], F32, tag="stkb")
                    nc.tensor.transpose(out=stkb[:], in_=o[:, t, :],
                                        identity=ident[:])
                    rhs = tailp.tile([128, 128], F32, tag="rhs")
                    nc.scalar.copy(out=rhs[0:D, :], in_=stka[:])
                    nc.scalar.copy(out=rhs[D:128, :], in_=stkb[:])
                    pre = psp.tile([D, 128], F32, tag="pre")
                    nc.tensor.matmul(out=pre[:], lhsT=uw_sb[:], rhs=rhs[:],
                                     start=True, stop=True)
                    lr = tailp.tile([D, 128], F32, tag="lr")
                    nc.vector.tensor_scalar(out=lr[:], in0=pre[:],
                                            scalar1=vb_sb[:], scalar2=None,
                                            op0=mybir.AluOpType.add)
                    lr2 = tailp.tile([D, 128], F32, tag="lr2")
                    nc.vector.tensor_scalar_mul(lr2[:], lr[:], 0.2)
                    nc.vector.tensor_tensor(out=lr[:], in0=lr[:], in1=lr2[:],
                                            op=mybir.AluOpType.max)
                    vout = psp.tile([1, 128], F32, tag="vout")
                    nc.tensor.matmul(out=vout[:], lhsT=v_sb[:], rhs=lr[:],
                                     start=True, stop=True)
                    vsb = tailp.tile([1, 128], F32, tag="vsb")
                    nc.scalar.copy(out=vsb[:], in_=vout[:])
                    off = b * pb + t * 128
                    nc.sync.dma_start(
                        out=t_out[off: off + 128].rearrange("(o n) -> o n", o=1),
                        in_=vsb[:])
    _fix_swdge_queue_nums(nc)
    nc.compile()
    return nc


def _fix_swdge_queue_nums(nc):
    """Align dma_gather queue_num with Tile's DMASW sem-lane rotation.

    Tile assigns SWDGE completion sems round-robin (lane = ordinal % 8) over
    Pool-engine DMA insts in final scheduled order; a sem lane must only ever
    be updated from one SWDGE queue, so set queue = lane % num_queues.
    """
    from concourse import bass_isa, mybir as mb
    ctr = 0
    for bb in nc.m.functions[0].blocks:
        for inst in bb.instructions:
            if isinstance(inst, bass_isa.AnyDMAInstruction) \
                    and inst.engine == mb.EngineType.Pool \
                    and not isinstance(inst, bass_isa.UserSyncedRemoteDMADescs):
                lane = ctr % 8
                ctr += 1
                if isinstance(inst, mb.InstDMAGatherAnt):
                    inst.queue_num = lane % 4


def _build_in_maps(pl, embedding_user, embedding_item, W_w, W_b, U_w, U_b,
                   b, v):
    uw = np.concatenate([U_w.T, W_w.T], axis=0).astype(np.float32).copy()
    vb = (U_b + W_b + b.reshape(-1)).astype(np.float32).reshape(D, 1).copy()
    vv = v.astype(np.float32).reshape(D, 1).copy()
    ident = np.eye(128, dtype=np.float32)
    ut = np.ascontiguousarray(embedding_user, dtype=np.float32)
    it = np.ascontiguousarray(embedding_item, dtype=np.float32)
    in_maps = []
    for (p1u, p1i, p2) in pl["cores"]:
        in_maps.append({
            "ut": ut, "it": it,
            "p1u": np.stack([_wrap16(r) for r in p1u]),
            "p1i": np.stack([_wrap16(r) for r in p1i]),
            "p2": np.stack([_wrap16(r) for r in p2]),
            "uw": uw, "vb": vb, "v": vv, "ident": ident,
        })
    return in_maps


def kernel(users, items, sampled_user, embedding_user, embedding_item,
           W_w, W_b, U_w, U_b, b, v):
    users = np.asarray(users).astype(np.int64)
    items = np.asarray(items).astype(np.int64)
    sampled_user = np.asarray(sampled_user)
    adjidx = np.asarray(sampled_user)[users]          # [N, S] host index prep
    npc = N_PAIRS // N_CORES
    pl = plan(users, items, adjidx, N_CORES, npc, nb=8)
    nc = build_program(pl)
    in_maps = _build_in_maps(pl, embedding_user, embedding_item,
                             W_w, W_b, U_w, U_b, b, v)
    res = run_bass_kernel_spmd(nc, in_maps, core_ids=list(range(N_CORES)))
    out = np.concatenate([r["out"] for r in res.results])
    return out.astype(np.float32)
